# revision 80
# baseline (speedup 1.0000x reference)
"""Trainium2 Bass kernel for nn_AttentionBlock (GroupNorm + single-head attention + residual).

Reference computation (b=4, c=256, h=w=64, n=h*w=4096):
    xn = GroupNorm(x, groups=8) * gamma + beta          # [b,c,n]
    q/k/v = w{q,k,v} @ xn + b{q,k,v}                    # 1x1 conv = channel matmul
    S = (q^T k) / sqrt(c);  P = softmax(S, axis=-1)     # [b,n,n]
    out = wp @ (v @ P^T) + bp + x

Sharding: pure data parallel, no collectives. Core p = 2*b + h handles batch b
and query half h (2048 queries), computing GroupNorm stats + keys/values for
its batch redundantly with its pair core. The input x is ROLLED on host by
h*2048 columns so queries are always columns 0:2048 of the core's xb (softmax
is permutation-invariant over keys, GN over spatial).

v2 math (fp8e4m3 DoubleRow matmuls, one 256-deep contraction per instruction):
  - Host precomputes M2 = wq^T wk, transposed wv/wp layouts, cbp = wp@bv+bp.
  - Device: GN stats -> per-channel A,B; xn8 = fp8(A*x + B) explicitly.
  - KS = M2^T-layout @ xn8 (keys premultiplied; no Q tensor needed).
  - S tile = KS^T xn8 in fp8 DoubleRow; exp via scalar engine with a uniform
    bias of -2.5 inside the exp (cancels in softmax; keeps exp < 448 = e4m3
    max). Softmax denominator accumulated on the PE: a ones-vector fp8
    matmul per key-chunk pair accumulates sum(exp) in PSUM - no DVE adds.
  - PV accumulates in PSUM over 16 DoubleRow pairs; normalize by 1/den
    (reciprocal_approx_fast + PE ones-outer-product broadcast), project with
    fp8 wp, add cbp + residual, DMA out.
Quantization error is dominated by fp8 (~3.6% RMS per tensor) on the
attention path only; the output is residual-dominated so the measured
rel err lands ~6e-3 (gate 2e-2). Validated against numpy mock.
"""

import numpy as np

P = 128
C = 256
HW = 4096
NQ = 2048
QB = 512
G = 8
EPS = 1e-5
NCORES = 8
NMB = HW // P     # 32 key chunks
NPAIR = NMB // 2  # 16 DoubleRow pairs
NQB = NQ // QB    # 4 query blocks
G2 = G // 2       # groups per channel half

_cache = {}


def _build():
    import concourse.bass as bass
    import concourse.mybir as mybir
    import concourse.tile as tile
    from concourse import bacc
    from concourse.tile_rust import add_dep_helper

    F32 = mybir.dt.float32
    FR = mybir.dt.float32r
    F8 = mybir.dt.float8e4
    AF = mybir.ActivationFunctionType
    OP = mybir.AluOpType
    DR = mybir.MatmulPerfMode.DoubleRow

    nc = bacc.Bacc("TRN2", target_bir_lowering=False, debug=False,
                   num_devices=NCORES)

    BF16 = mybir.dt.bfloat16
    # x in partition-major [p, cc, n] layout: bf16 copy for stats + xn (fast
    # 8KB-contiguous-per-partition DMA), f32 copy for the residual (loads
    # lazily off the critical path)
    xh = nc.dram_tensor("xh", [P, 2 * HW], BF16, kind="ExternalInput")
    xf = nc.dram_tensor("xf", [P, 2 * HW], F32, kind="ExternalInput")
    m2t_d = nc.dram_tensor("m2t", [P, 2 * C], F32, kind="ExternalInput")
    # M3 = wp @ wv precomputed on host: W = M3 @ xn are the per-key value
    # vectors already projected by wp, so PV accumulation directly yields the
    # projected output and the separate proj matmul stage disappears
    m3t_d = nc.dram_tensor("m3t", [P, 2 * C], F32, kind="ExternalInput")
    # packed small constants: [:, 0:2]=gamma, [2:4]=beta, [4:6]=cbp,
    # [6:22]=group indicator (cc-major, value 1/32)
    cst_d = nc.dram_tensor("cst", [P, 22], F32, kind="ExternalInput")
    # transposed group indicator (value 1.0): [g%4, cc*128+i] — the pattern
    # repeats per cc half, so the per-cc broadcast matmuls stay aligned at
    # partitions 0..4
    cstT_d = nc.dram_tensor("cstT", [G // 2, C], F32, kind="ExternalInput")
    y = nc.dram_tensor("y", [C, NQ], F32, kind="ExternalOutput")

    xh_t = xh.rearrange("p (cc n) -> p cc n", cc=2)
    xf_t = xf.rearrange("p (cc n) -> p cc n", cc=2)
    y_t = y.rearrange("(cc p) n -> p cc n", p=P)

    with tile.TileContext(nc) as tc:
        with (
            tc.tile_pool(name="persist", bufs=1) as pers,
            tc.tile_pool(name="tmp", bufs=3) as tmp,
            tc.tile_pool(name="pt", bufs=3) as ptp,
            tc.tile_pool(name="rd", bufs=2) as rdp,
            tc.tile_pool(name="outp", bufs=4) as outp,
        ):
            # ---------------- small constants ----------------
            cst = pers.tile([P, 22], F32)
            nc.sync.dma_start(out=cst, in_=cst_d[:, :])
            cstT = pers.tile([G // 2, C], F32)
            nc.sync.dma_start(out=cstT, in_=cstT_d[:, :])
            gm = cst[:, 0:2]
            bt = cst[:, 2:4]
            cbp = cst[:, 4:6]
            ind = cst[:, 6:22].rearrange("p (cc g) -> p cc g", cc=2)

            # all-ones fp8 weights, [P, 2, 128]: the den matmul broadcasts
            # sum(exp) to every output partition (stream cost is free-size
            # only), so no separate 1/den broadcast is needed
            ones32 = pers.tile([P, 2, P], F32)
            nc.vector.memset(ones32, 1.0)
            ones8 = pers.tile([P, 2, P], F8)
            nc.vector.tensor_copy(ones8, ones32)
            nbias = pers.tile([P, 1], F32)
            nc.vector.memset(nbias, -2.5)

            # ---------------- input DMAs (priority-staged) ----------------
            # bf16 x first (it gates everything), staged in 3 pieces so stats
            # overlap the transfer; then weights (needed ~10us later); then
            # f32 x (residual, needed much later). Staging via explicit deps —
            # the DMA engines round-robin ALL outstanding transfers otherwise.
            X = pers.tile([P, 2, HW], BF16)
            xh_a = nc.scalar.dma_start(out=X[:, 0, :], in_=xh_t[:, 0, :])
            xh_b = nc.sync.dma_start(out=X[:, 1, 0:NQ], in_=xh_t[:, 1, 0:NQ])
            xh_c = nc.scalar.dma_start(out=X[:, 1, NQ:HW], in_=xh_t[:, 1, NQ:HW])
            add_dep_helper(xh_b.ins, xh_a.ins, True, "stage bf16 x: cc1h0 after cc0")
            add_dep_helper(xh_c.ins, xh_b.ins, True, "stage bf16 x: cc1h1 after cc1h0")

            m2w = pers.tile([P, 2, C], F32)
            wdma = [nc.scalar.dma_start(
                out=m2w, in_=m2t_d.rearrange("p (s c) -> p s c", s=2))]
            m3w = pers.tile([P, 2, C], F32)
            wdma.append(nc.sync.dma_start(
                out=m3w, in_=m3t_d.rearrange("p (s c) -> p s c", s=2)))
            for w in wdma:
                add_dep_helper(w.ins, xh_c.ins, True, "weights after bf16 x")
            Xf = pers.tile([P, 2, HW], F32)
            for cc in range(2):
                xfd = nc.sync.dma_start(out=Xf[:, cc, :], in_=xf_t[:, cc, :])
                for w in wdma:
                    add_dep_helper(xfd.ins, w.ins, True,
                                   "f32 residual copy last")

            # fp8 weight casts (SBUF->SBUF: Pool engine; it cannot touch PSUM)
            m2_8 = pers.tile([P, 2, C], F8)
            nc.gpsimd.tensor_copy(m2_8, m2w)
            m3_8 = pers.tile([P, 2, C], F8)
            nc.gpsimd.tensor_copy(m3_8, m3w)

            # ---------------- GroupNorm stats -> A, B ----------------
            # st2[:, 0] = sum_n x, st2[:, 1] = sum_n x^2 per channel; the
            # group indicator carries 1/(32*4096) so the gst matmul yields
            # group means directly. sum_x on DVE (bf16 2x reduce), sum_x2 on
            # the otherwise-idle Act engine (Square + accum_out).
            with tc.tile_pool(name="ps_prep", bufs=2, space="PSUM") as psp:
                st2s = [tmp.tile([P, 2], F32, tag=f"st2_{cc}",
                                 name=f"st2_{cc}") for cc in range(2)]
                trash = [tmp.tile([P, HW], BF16, tag=f"sqtrash{cc}",
                                  name=f"sqtrash{cc}") for cc in range(2)]
                # per staged piece (as each DMA lands): Act does sum(x^2) via
                # Square+accum_out, DVE does sum(x)
                sq_b = tmp.tile([P, 2], F32, tag="sq_b")
                nc.scalar.activation(out=trash[0], in_=X[:, 0, :],
                                     func=AF.Square, accum_out=st2s[0][:, 1:2])
                nc.vector.reduce_sum(out=st2s[0][:, 0:1], in_=X[:, 0, :],
                                     axis=mybir.AxisListType.X)
                nc.scalar.activation(out=trash[1][:, 0:NQ], in_=X[:, 1, 0:NQ],
                                     func=AF.Square, accum_out=st2s[1][:, 1:2])
                nc.vector.reduce_sum(out=st2s[1][:, 0:1], in_=X[:, 1, 0:NQ],
                                     axis=mybir.AxisListType.X)
                nc.scalar.activation(out=trash[1][:, NQ:HW], in_=X[:, 1, NQ:HW],
                                     func=AF.Square, accum_out=sq_b[:, 1:2])
                nc.vector.reduce_sum(out=sq_b[:, 0:1], in_=X[:, 1, NQ:HW],
                                     axis=mybir.AxisListType.X)
                nc.vector.tensor_add(st2s[1], st2s[1], sq_b)
                # per-cc chain: groups 0-3 live wholly in cc0 and 4-7 in cc1,
                # so cc0's full stats->A/B chain (including the one-time
                # Ln/Exp table loads) completes while cc1 is still loading.
                # rstd = exp(-0.5*ln(var+eps)): Ln/Exp/Square share one
                # activation table -> no table reloads around the exp stream.
                eps_t = pers.tile([G2, 1], F32)
                nc.vector.memset(eps_t, EPS)
                A = pers.tile([P, 2], F32)
                Bv = pers.tile([P, 2], F32)
                for cc in range(2):
                    gst = psp.tile([G2, 2], F32, tag=f"gst{cc}",
                                   name=f"gst{cc}")
                    nc.tensor.matmul(gst, ind[:, cc, 4 * cc:4 * cc + 4],
                                     st2s[cc], start=True, stop=True)
                    gss = pers.tile([G2, 2], F32, name=f"gss{cc}")
                    nc.vector.tensor_copy(gss, gst)
                    varg = pers.tile([G2, 1], F32, name=f"varg{cc}")
                    nc.vector.tensor_mul(varg, gss[:, 0:1], gss[:, 0:1])
                    nc.vector.tensor_tensor(varg, gss[:, 1:2], varg,
                                            OP.subtract)
                    lnv = pers.tile([G2, 1], F32, name=f"lnv{cc}")
                    nc.scalar.activation(out=lnv, in_=varg, func=AF.Ln,
                                         bias=eps_t)
                    gsb = pers.tile([G2, 2], F32, name=f"gsb{cc}")
                    nc.vector.tensor_copy(gsb[:, 0:1], gss[:, 0:1])
                    nc.scalar.activation(out=gsb[:, 1:2], in_=lnv, func=AF.Exp,
                                         scale=-0.5)
                    bc = psp.tile([P, 2], F32, tag="bc", name=f"bc{cc}")
                    nc.tensor.matmul(bc, cstT[:, cc * P:(cc + 1) * P], gsb,
                                     start=True, stop=True)
                    nc.vector.tensor_mul(A[:, cc:cc + 1], bc[:, 1:2], gm[:, cc:cc + 1])
                    nc.vector.tensor_mul(Bv[:, cc:cc + 1], bc[:, 0:1], A[:, cc:cc + 1])
                    nc.vector.tensor_tensor(Bv[:, cc:cc + 1], bt[:, cc:cc + 1],
                                            Bv[:, cc:cc + 1], OP.subtract)

            # ---------------- normalized fp8 activations ----------------
            # xn8 = fp8(A*x + B): chunks emitted just-in-time inside the qb0
            # pair loop (below) so early chunks aren't queued behind late ones
            xn8 = pers.tile([P, 2, HW], F8)

            def emit_xn(ch):
                cs = slice(512 * ch, 512 * (ch + 1))
                eng = nc.vector if ch % 2 == 0 else nc.gpsimd
                for cc in range(2):
                    eng.tensor_scalar(out=xn8[:, cc, cs], in0=X[:, cc, cs],
                                      scalar1=A[:, cc:cc + 1],
                                      scalar2=Bv[:, cc:cc + 1],
                                      op0=OP.mult, op1=OP.add)

            # ---------------- attention ----------------
            # Q8 = M2^T-layout @ xn over the 2048 query columns only;
            # S[k, q] = sum_c xn[c, k] Q[c, q] needs no key-side premultiply.
            # WT8[k, o] = projected values (M3 @ xn)^T per key.
            Q8 = pers.tile([P, 2, NQ], F8)
            WT8 = pers.tile([P, NMB, C], F8)

            with (
                tc.tile_pool(name="ps_s", bufs=2, space="PSUM") as pss,
                tc.tile_pool(name="ps_pv", bufs=2, space="PSUM") as pspv,
                tc.tile_pool(name="ps_vt", bufs=1, space="PSUM") as psvt,
                tc.tile_pool(name="ps_den", bufs=1, space="PSUM") as psd,
            ):
                def emit_q(qb):
                    # Q cols for query block qb
                    cs = slice(QB * qb, QB * (qb + 1))
                    qps = pss.tile([P, 2, QB], F32, tag="s", name=f"q_{qb}")
                    for co in range(2):
                        nc.tensor.matmul(qps[:, co, :],
                                         m2_8[:, :, co * P:(co + 1) * P],
                                         xn8[:, :, cs],
                                         start=True, stop=True, perf_mode=DR)
                    nc.vector.tensor_copy(Q8[:, 0, cs], qps[:, 0, :])
                    nc.vector.tensor_copy(Q8[:, 1, cs], qps[:, 1, :])

                def emit_wt(jv):
                    # WT key chunks 2jv, 2jv+1
                    wtps = psvt.tile([P, 2, C], F32, tag="vt", name=f"wt_{jv}")
                    for half in range(2):
                        m = 2 * jv + half
                        nc.tensor.matmul(wtps[:, half, :],
                                         xn8[:, :, P * m:P * (m + 1)],
                                         m3_8, start=True, stop=True,
                                         perf_mode=DR)
                    nc.vector.tensor_copy(WT8[:, 2 * jv:2 * jv + 2, :], wtps)

                def emit_s(qb, j):
                    qs = slice(QB * qb, QB * (qb + 1))
                    sp = pss.tile([P, 2, QB], F32, tag="s", name=f"s_{qb}_{j}")
                    for half in range(2):
                        m = 2 * j + half
                        nc.tensor.matmul(sp[:, half, :],
                                         xn8[:, :, P * m:P * (m + 1)],
                                         Q8[:, :, qs],
                                         start=True, stop=True, perf_mode=DR)
                    return sp

                def emit_pv(qb, j, sp, pw0, pw1, den):
                    pt = ptp.tile([P, 2, QB], F8, tag="pt", name=f"pt_{qb}_{j}")
                    nc.scalar.activation(out=pt, in_=sp, func=AF.Exp,
                                         scale=0.0625, bias=nbias)
                    nc.tensor.matmul(pw0, WT8[:, 2 * j:2 * j + 2, 0:P], pt,
                                     start=(j == 0), stop=(j == NPAIR - 1),
                                     perf_mode=DR)
                    nc.tensor.matmul(pw1, WT8[:, 2 * j:2 * j + 2, P:C], pt,
                                     start=(j == 0), stop=(j == NPAIR - 1),
                                     perf_mode=DR)
                    nc.tensor.matmul(den, ones8, pt,
                                     start=(j == 0), stop=(j == NPAIR - 1),
                                     perf_mode=DR)

                def emit_out(qb, pw0, pw1, den):
                    # 1/den (broadcast across partitions by the ones matmul),
                    # then y = pw*rd + cbp + x straight out of the PSUMs
                    qs = slice(QB * qb, QB * (qb + 1))
                    rd = rdp.tile([P, QB], F32, tag="rd", name=f"rd_{qb}")
                    nc.vector.reciprocal_approx_fast(out=rd, in_=den)
                    for oc, pw in ((0, pw0), (1, pw1)):
                        ou = outp.tile([P, QB], F32, tag="out",
                                       name=f"ou_{qb}_{oc}")
                        nc.vector.tensor_mul(ou, pw, rd)
                        nc.vector.tensor_scalar_add(out=ou, in0=ou,
                                                    scalar1=cbp[:, oc:oc + 1])
                        nc.gpsimd.tensor_add(ou, ou, Xf[:, oc, qs])
                        nc.sync.dma_start(out=y_t[:, oc, qs], in_=ou)

                # ---- software-pipelined attention stream ----
                # qb0 additionally produces xn8 chunks (two ahead) and VT (one
                # pair ahead) inline; Q for the next qb block is emitted
                # mid-stream; epilogue for the previous qb is staggered into
                # the next qb's pair stream.
                emit_xn(0)
                emit_xn(1)
                emit_q(0)
                emit_wt(0)
                s_q = []
                pvs = {}
                for qb in range(NQB):
                    for j in range(NPAIR):
                        if qb == 0:
                            if j % 2 == 0 and j // 2 + 2 < 8:
                                emit_xn(j // 2 + 2)
                            if j + 1 < NPAIR:
                                emit_wt(j + 1)
                        if j == 8 and qb + 1 < NQB:
                            emit_q(qb + 1)
                        if j == 0:
                            pvs[qb] = (
                                pspv.tile([P, QB], F32, tag="pv",
                                          name=f"pv0_{qb}"),
                                pspv.tile([P, QB], F32, tag="pv",
                                          name=f"pv1_{qb}"),
                                psd.tile([P, QB], F32, tag="den",
                                         name=f"den_{qb}"),
                            )
                        s_q.append((qb, j, emit_s(qb, j)))
                        if len(s_q) > 1:
                            pqb, pj, psp_ = s_q.pop(0)
                            emit_pv(pqb, pj, psp_, *pvs[pqb])
                            if pj == NPAIR - 1:
                                emit_out(pqb, *pvs[pqb])
                # tail: drain last pair + final epilogue
                for pqb, pj, psp_ in s_q:
                    emit_pv(pqb, pj, psp_, *pvs[pqb])
                    if pj == NPAIR - 1:
                        emit_out(pqb, *pvs[pqb])

    nc.compile()
    return nc


def _get_nc():
    if "nc" not in _cache:
        _cache["nc"] = _build()
    return _cache["nc"]


def _host_prep(inputs):
    """Precompute weight layouts + packed constants (all fp32)."""
    wq = np.asarray(inputs["wq"], np.float32)
    wk = np.asarray(inputs["wk"], np.float32)
    wv = np.asarray(inputs["wv"], np.float32)
    wp = np.asarray(inputs["wp"], np.float32)
    M2 = wq.T @ wk  # [c'(q-side), c(k-side)]
    # lhsT layout [p, s, i] = M[i, p + 128 s], flattened to [P, 2*C]
    def lay(m):
        return np.ascontiguousarray(
            m.T.reshape(2, P, C).transpose(1, 0, 2).reshape(P, 2 * C))
    cbp = wp @ np.asarray(inputs["bv"], np.float32) + np.asarray(
        inputs["bp"], np.float32)
    cst = np.zeros((P, 22), np.float32)
    for i, v in enumerate((inputs["gn_gamma"], inputs["gn_beta"], cbp)):
        cst[:, 2 * i:2 * i + 2] = np.asarray(v, np.float32).reshape(2, P).T
    # ind[p, cc, g]: group-averaging weight; st2 holds per-channel SUMS over
    # n, so fold the full 1/(32*4096) mean divisor in here
    for cc in range(2):
        for j in range(4):
            g = cc * 4 + j
            cst[32 * j:32 * (j + 1), 6 + cc * G + g] = 1.0 / (32.0 * HW)
    # [g%4, cc*128+i]: same indicator pattern in each cc half
    cstT = np.zeros((G // 2, C), np.float32)
    for cc in range(2):
        for j in range(4):
            cstT[j, cc * P + 32 * j:cc * P + 32 * (j + 1)] = 1.0
    return {
        "m2t": lay(M2.T),
        "m3t": lay(wp @ wv),
        "cst": cst,
        "cstT": cstT,
    }


def _core_x(x, b, h):
    """Per-core x tensors: rolled so queries are cols 0:NQ, then laid out
    [p, cc, n] flattened to [P, 2*HW], in bf16 (stats+xn) and f32 (residual)."""
    import ml_dtypes

    xr = np.roll(x[b], -h * NQ, axis=1)           # [C, HW]
    xl = np.ascontiguousarray(
        xr.reshape(2, P, HW).transpose(1, 0, 2).reshape(P, 2 * HW))
    return {
        "xh": xl.astype(ml_dtypes.bfloat16),
        "xf": xl,
    }


def kernel(**inputs):
    from concourse.bass_utils import run_bass_kernel_spmd

    nc = _get_nc()
    x = np.ascontiguousarray(np.asarray(inputs["x"], dtype=np.float32)
                             ).reshape(4, C, HW)
    common = _host_prep(inputs)
    in_maps = []
    for p in range(NCORES):
        b, h = divmod(p, 2)
        m = dict(common)
        m.update(_core_x(x, b, h))
        in_maps.append(m)
    res = run_bass_kernel_spmd(nc, in_maps, list(range(NCORES)))
    out = np.empty((4, C, HW), np.float32)
    for p in range(NCORES):
        b, h = divmod(p, 2)
        out[b, :, h * NQ:(h + 1) * NQ] = res.results[p]["y"]
    return out.reshape(4, C, 64, 64)


# revision 87
# speedup vs baseline: 1.0156x; 1.0156x over previous
"""Trainium2 Bass kernel for nn_AttentionBlock (GroupNorm + single-head attention + residual).

Reference computation (b=4, c=256, h=w=64, n=h*w=4096):
    xn = GroupNorm(x, groups=8) * gamma + beta          # [b,c,n]
    q/k/v = w{q,k,v} @ xn + b{q,k,v}                    # 1x1 conv = channel matmul
    S = (q^T k) / sqrt(c);  P = softmax(S, axis=-1)     # [b,n,n]
    out = wp @ (v @ P^T) + bp + x

Sharding: pure data parallel, no collectives. Core p = 2*b + h handles batch b
and query half h (2048 queries), computing GroupNorm stats + keys/values for
its batch redundantly with its pair core. The input x is ROLLED on host by
h*2048 columns so queries are always columns 0:2048 of the core's xb (softmax
is permutation-invariant over keys, GN over spatial).

v2 math (fp8e4m3 DoubleRow matmuls, one 256-deep contraction per instruction):
  - Host precomputes M2 = wq^T wk, transposed wv/wp layouts, cbp = wp@bv+bp.
  - Device: GN stats -> per-channel A,B; xn8 = fp8(A*x + B) explicitly.
  - KS = M2^T-layout @ xn8 (keys premultiplied; no Q tensor needed).
  - S tile = KS^T xn8 in fp8 DoubleRow; exp via scalar engine with a uniform
    bias of -2.5 inside the exp (cancels in softmax; keeps exp < 448 = e4m3
    max). Softmax denominator accumulated on the PE: a ones-vector fp8
    matmul per key-chunk pair accumulates sum(exp) in PSUM - no DVE adds.
  - PV accumulates in PSUM over 16 DoubleRow pairs; normalize by 1/den
    (reciprocal_approx_fast + PE ones-outer-product broadcast), project with
    fp8 wp, add cbp + residual, DMA out.
Quantization error is dominated by fp8 (~3.6% RMS per tensor) on the
attention path only; the output is residual-dominated so the measured
rel err lands ~6e-3 (gate 2e-2). Validated against numpy mock.
"""

import numpy as np

P = 128
C = 256
HW = 4096
NQ = 2048
QB = 512
G = 8
EPS = 1e-5
NCORES = 8
NMB = HW // P     # 32 key chunks
NPAIR = NMB // 2  # 16 DoubleRow pairs
NQB = NQ // QB    # 4 query blocks
G2 = G // 2       # groups per channel half

_cache = {}


def _build():
    import concourse.bass as bass
    import concourse.mybir as mybir
    import concourse.tile as tile
    from concourse import bacc
    from concourse.tile_rust import add_dep_helper

    F32 = mybir.dt.float32
    FR = mybir.dt.float32r
    F8 = mybir.dt.float8e4
    AF = mybir.ActivationFunctionType
    OP = mybir.AluOpType
    DR = mybir.MatmulPerfMode.DoubleRow

    nc = bacc.Bacc("TRN2", target_bir_lowering=False, debug=False,
                   num_devices=NCORES)

    BF16 = mybir.dt.bfloat16
    # x in partition-major [p, cc, n] layout: bf16 copy for stats + xn (fast
    # 8KB-contiguous-per-partition DMA), f32 copy for the residual (loads
    # lazily off the critical path)
    xh = nc.dram_tensor("xh", [P, 2 * HW], BF16, kind="ExternalInput")
    xf = nc.dram_tensor("xf", [P, 2 * HW], F32, kind="ExternalInput")
    m2t_d = nc.dram_tensor("m2t", [P, 2 * C], F32, kind="ExternalInput")
    # M3 = wp @ wv precomputed on host: W = M3 @ xn are the per-key value
    # vectors already projected by wp, so PV accumulation directly yields the
    # projected output and the separate proj matmul stage disappears
    m3t_d = nc.dram_tensor("m3t", [P, 2 * C], F32, kind="ExternalInput")
    # packed small constants: [:, 0:2]=gamma, [2:4]=beta, [4:6]=cbp,
    # [6:22]=group indicator (cc-major, value 1/32)
    cst_d = nc.dram_tensor("cst", [P, 22], F32, kind="ExternalInput")
    # transposed group indicator (value 1.0): [g, c]
    cstT_d = nc.dram_tensor("cstT", [G, C], F32, kind="ExternalInput")
    y = nc.dram_tensor("y", [C, NQ], F32, kind="ExternalOutput")

    xh_t = xh.rearrange("p (cc n) -> p cc n", cc=2)
    xf_t = xf.rearrange("p (cc n) -> p cc n", cc=2)
    y_t = y.rearrange("(cc p) n -> p cc n", p=P)

    with tile.TileContext(nc) as tc:
        with (
            tc.tile_pool(name="persist", bufs=1) as pers,
            tc.tile_pool(name="tmp", bufs=3) as tmp,
            tc.tile_pool(name="pt", bufs=3) as ptp,
            tc.tile_pool(name="rd", bufs=2) as rdp,
            tc.tile_pool(name="outp", bufs=4) as outp,
        ):
            # ---------------- small constants ----------------
            cst = pers.tile([P, 22], F32)
            nc.sync.dma_start(out=cst, in_=cst_d[:, :])
            cstT = pers.tile([G, C], F32)
            nc.sync.dma_start(out=cstT, in_=cstT_d[:, :])
            gm = cst[:, 0:2]
            bt = cst[:, 2:4]
            cbp = cst[:, 4:6]
            ind = cst[:, 6:22].rearrange("p (cc g) -> p cc g", cc=2)

            # all-ones fp8 weights, [P, 2, 128]: the den matmul broadcasts
            # sum(exp) to every output partition (stream cost is free-size
            # only), so no separate 1/den broadcast is needed
            ones32 = pers.tile([P, 2, P], F32)
            nc.vector.memset(ones32, 1.0)
            ones8 = pers.tile([P, 2, P], F8)
            nc.vector.tensor_copy(ones8, ones32)
            nbias = pers.tile([P, 1], F32)
            nc.vector.memset(nbias, -2.5)

            # ---------------- input DMAs (priority-staged) ----------------
            # bf16 x first (it gates everything), staged in 3 pieces so stats
            # overlap the transfer; then weights (needed ~10us later); then
            # f32 x (residual, needed much later). Staging via explicit deps —
            # the DMA engines round-robin ALL outstanding transfers otherwise.
            X = pers.tile([P, 2, HW], BF16)
            xh_a = nc.scalar.dma_start(out=X[:, 0, :], in_=xh_t[:, 0, :])
            xh_b = nc.sync.dma_start(out=X[:, 1, 0:NQ], in_=xh_t[:, 1, 0:NQ])
            xh_c = nc.scalar.dma_start(out=X[:, 1, NQ:HW], in_=xh_t[:, 1, NQ:HW])
            add_dep_helper(xh_b.ins, xh_a.ins, True, "stage bf16 x: cc1h0 after cc0")
            add_dep_helper(xh_c.ins, xh_b.ins, True, "stage bf16 x: cc1h1 after cc1h0")

            m2w = pers.tile([P, 2, C], F32)
            wdma = [nc.scalar.dma_start(
                out=m2w, in_=m2t_d.rearrange("p (s c) -> p s c", s=2))]
            m3w = pers.tile([P, 2, C], F32)
            wdma.append(nc.sync.dma_start(
                out=m3w, in_=m3t_d.rearrange("p (s c) -> p s c", s=2)))
            for w in wdma:
                add_dep_helper(w.ins, xh_c.ins, True, "weights after bf16 x")
            Xf = pers.tile([P, 2, HW], F32)
            for cc in range(2):
                xfd = nc.sync.dma_start(out=Xf[:, cc, :], in_=xf_t[:, cc, :])
                for w in wdma:
                    add_dep_helper(xfd.ins, w.ins, True,
                                   "f32 residual copy last")

            # fp8 weight casts (SBUF->SBUF: Pool engine; it cannot touch PSUM)
            m2_8 = pers.tile([P, 2, C], F8)
            nc.gpsimd.tensor_copy(m2_8, m2w)
            m3_8 = pers.tile([P, 2, C], F8)
            nc.gpsimd.tensor_copy(m3_8, m3w)

            # ---------------- GroupNorm stats -> A, B ----------------
            # st2[:, 0] = sum_n x, st2[:, 1] = sum_n x^2 per channel; the
            # group indicator carries 1/(32*4096) so the gst matmul yields
            # group means directly. sum_x on DVE (bf16 2x reduce), sum_x2 on
            # the otherwise-idle Act engine (Square + accum_out).
            with tc.tile_pool(name="ps_prep", bufs=2, space="PSUM") as psp:
                st2s = [tmp.tile([P, 2], F32, tag=f"st2_{cc}",
                                 name=f"st2_{cc}") for cc in range(2)]
                trash = [tmp.tile([P, HW], BF16, tag=f"sqtrash{cc}",
                                  name=f"sqtrash{cc}") for cc in range(2)]
                # per staged piece (as each DMA lands): Act does sum(x^2) via
                # Square+accum_out, DVE does sum(x)
                sq_b = tmp.tile([P, 2], F32, tag="sq_b")
                nc.scalar.activation(out=trash[0], in_=X[:, 0, :],
                                     func=AF.Square, accum_out=st2s[0][:, 1:2])
                nc.vector.reduce_sum(out=st2s[0][:, 0:1], in_=X[:, 0, :],
                                     axis=mybir.AxisListType.X)
                nc.scalar.activation(out=trash[1][:, 0:NQ], in_=X[:, 1, 0:NQ],
                                     func=AF.Square, accum_out=st2s[1][:, 1:2])
                nc.vector.reduce_sum(out=st2s[1][:, 0:1], in_=X[:, 1, 0:NQ],
                                     axis=mybir.AxisListType.X)
                nc.scalar.activation(out=trash[1][:, NQ:HW], in_=X[:, 1, NQ:HW],
                                     func=AF.Square, accum_out=sq_b[:, 1:2])
                nc.vector.reduce_sum(out=sq_b[:, 0:1], in_=X[:, 1, NQ:HW],
                                     axis=mybir.AxisListType.X)
                nc.vector.tensor_add(st2s[1], st2s[1], sq_b)
                # group stats: gst matmuls split per cc (each group's channels
                # live in one cc half) so cc0's sum can issue early; the
                # scalar chain itself stays unified to minimize Act
                # table swaps (Ln and Exp sit in different tables).
                # rstd = exp(-0.5*ln(var+eps)).
                eps_t = pers.tile([G, 1], F32)
                nc.vector.memset(eps_t, EPS)
                gst = psp.tile([G, 2], F32, tag="gst")
                for cc in range(2):
                    nc.tensor.matmul(gst, ind[:, cc, :], st2s[cc],
                                     start=(cc == 0), stop=(cc == 1))
                gss = pers.tile([G, 2], F32)
                nc.vector.tensor_copy(gss, gst)
                varg = pers.tile([G, 1], F32)
                nc.vector.tensor_mul(varg, gss[:, 0:1], gss[:, 0:1])
                nc.vector.tensor_tensor(varg, gss[:, 1:2], varg, OP.subtract)
                lnv = pers.tile([G, 1], F32)
                nc.scalar.activation(out=lnv, in_=varg, func=AF.Ln, bias=eps_t)
                gsb = pers.tile([G, 2], F32)
                nc.vector.tensor_copy(gsb[:, 0:1], gss[:, 0:1])
                nc.scalar.activation(out=gsb[:, 1:2], in_=lnv, func=AF.Exp,
                                     scale=-0.5)
                A = pers.tile([P, 2], F32)
                Bv = pers.tile([P, 2], F32)
                for cc in range(2):
                    bc = psp.tile([P, 2], F32, tag="bc", name=f"bc{cc}")
                    nc.tensor.matmul(bc, cstT[:, cc * P:(cc + 1) * P], gsb,
                                     start=True, stop=True)
                    nc.vector.tensor_mul(A[:, cc:cc + 1], bc[:, 1:2], gm[:, cc:cc + 1])
                    nc.vector.tensor_mul(Bv[:, cc:cc + 1], bc[:, 0:1], A[:, cc:cc + 1])
                    nc.vector.tensor_tensor(Bv[:, cc:cc + 1], bt[:, cc:cc + 1],
                                            Bv[:, cc:cc + 1], OP.subtract)

            # ---------------- normalized fp8 activations ----------------
            # xn8 = fp8(A*x + B): chunks emitted just-in-time inside the qb0
            # pair loop (below) so early chunks aren't queued behind late ones
            xn8 = pers.tile([P, 2, HW], F8)

            def emit_xn(ch):
                cs = slice(512 * ch, 512 * (ch + 1))
                eng = nc.vector if ch % 2 == 0 else nc.gpsimd
                for cc in range(2):
                    eng.tensor_scalar(out=xn8[:, cc, cs], in0=X[:, cc, cs],
                                      scalar1=A[:, cc:cc + 1],
                                      scalar2=Bv[:, cc:cc + 1],
                                      op0=OP.mult, op1=OP.add)

            # ---------------- attention ----------------
            # Q8 = M2^T-layout @ xn over the 2048 query columns only;
            # S[k, q] = sum_c xn[c, k] Q[c, q] needs no key-side premultiply.
            # WT8[k, o] = projected values (M3 @ xn)^T per key.
            Q8 = pers.tile([P, 2, NQ], F8)
            WT8 = pers.tile([P, NMB, C], F8)

            with (
                tc.tile_pool(name="ps_s", bufs=2, space="PSUM") as pss,
                tc.tile_pool(name="ps_pv", bufs=2, space="PSUM") as pspv,
                tc.tile_pool(name="ps_vt", bufs=1, space="PSUM") as psvt,
                tc.tile_pool(name="ps_den", bufs=1, space="PSUM") as psd,
            ):
                def emit_q(qb):
                    # Q cols for query block qb
                    cs = slice(QB * qb, QB * (qb + 1))
                    qps = pss.tile([P, 2, QB], F32, tag="s", name=f"q_{qb}")
                    for co in range(2):
                        nc.tensor.matmul(qps[:, co, :],
                                         m2_8[:, :, co * P:(co + 1) * P],
                                         xn8[:, :, cs],
                                         start=True, stop=True, perf_mode=DR)
                    nc.vector.tensor_copy(Q8[:, 0, cs], qps[:, 0, :])
                    nc.vector.tensor_copy(Q8[:, 1, cs], qps[:, 1, :])

                def emit_wt(jv):
                    # WT key chunks 2jv, 2jv+1
                    wtps = psvt.tile([P, 2, C], F32, tag="vt", name=f"wt_{jv}")
                    for half in range(2):
                        m = 2 * jv + half
                        nc.tensor.matmul(wtps[:, half, :],
                                         xn8[:, :, P * m:P * (m + 1)],
                                         m3_8, start=True, stop=True,
                                         perf_mode=DR)
                    nc.vector.tensor_copy(WT8[:, 2 * jv:2 * jv + 2, :], wtps)

                def emit_s(qb, j):
                    qs = slice(QB * qb, QB * (qb + 1))
                    sp = pss.tile([P, 2, QB], F32, tag="s", name=f"s_{qb}_{j}")
                    for half in range(2):
                        m = 2 * j + half
                        nc.tensor.matmul(sp[:, half, :],
                                         xn8[:, :, P * m:P * (m + 1)],
                                         Q8[:, :, qs],
                                         start=True, stop=True, perf_mode=DR)
                    return sp

                def emit_pv(qb, j, sp, pw0, pw1, den):
                    pt = ptp.tile([P, 2, QB], F8, tag="pt", name=f"pt_{qb}_{j}")
                    nc.scalar.activation(out=pt, in_=sp, func=AF.Exp,
                                         scale=0.0625, bias=nbias)
                    nc.tensor.matmul(pw0, WT8[:, 2 * j:2 * j + 2, 0:P], pt,
                                     start=(j == 0), stop=(j == NPAIR - 1),
                                     perf_mode=DR)
                    nc.tensor.matmul(pw1, WT8[:, 2 * j:2 * j + 2, P:C], pt,
                                     start=(j == 0), stop=(j == NPAIR - 1),
                                     perf_mode=DR)
                    nc.tensor.matmul(den, ones8, pt,
                                     start=(j == 0), stop=(j == NPAIR - 1),
                                     perf_mode=DR)

                def emit_out(qb, pw0, pw1, den):
                    # 1/den (broadcast across partitions by the ones matmul),
                    # then y = pw*rd + cbp + x straight out of the PSUMs
                    qs = slice(QB * qb, QB * (qb + 1))
                    rd = rdp.tile([P, QB], F32, tag="rd", name=f"rd_{qb}")
                    nc.vector.reciprocal_approx_fast(out=rd, in_=den)
                    for oc, pw in ((0, pw0), (1, pw1)):
                        ou = outp.tile([P, QB], F32, tag="out",
                                       name=f"ou_{qb}_{oc}")
                        nc.vector.tensor_mul(ou, pw, rd)
                        nc.vector.tensor_scalar_add(out=ou, in0=ou,
                                                    scalar1=cbp[:, oc:oc + 1])
                        nc.gpsimd.tensor_add(ou, ou, Xf[:, oc, qs])
                        nc.sync.dma_start(out=y_t[:, oc, qs], in_=ou)

                # ---- software-pipelined attention stream ----
                # qb0 additionally produces xn8 chunks (two ahead) and VT (one
                # pair ahead) inline; Q for the next qb block is emitted
                # mid-stream; epilogue for the previous qb is staggered into
                # the next qb's pair stream.
                emit_xn(0)
                emit_xn(1)
                emit_q(0)
                emit_wt(0)
                s_q = []
                pvs = {}
                for qb in range(NQB):
                    for j in range(NPAIR):
                        if qb == 0:
                            if j % 2 == 0 and j // 2 + 2 < 8:
                                emit_xn(j // 2 + 2)
                            if j + 1 < NPAIR:
                                emit_wt(j + 1)
                        if j == 8 and qb + 1 < NQB:
                            emit_q(qb + 1)
                        if j == 0:
                            pvs[qb] = (
                                pspv.tile([P, QB], F32, tag="pv",
                                          name=f"pv0_{qb}"),
                                pspv.tile([P, QB], F32, tag="pv",
                                          name=f"pv1_{qb}"),
                                psd.tile([P, QB], F32, tag="den",
                                         name=f"den_{qb}"),
                            )
                        s_q.append((qb, j, emit_s(qb, j)))
                        # pop with lag 1 (lag 0 for the very first pair so the
                        # Act stream starts one S-step sooner)
                        if len(s_q) > 1 or (qb == 0 and j == 0):
                            pqb, pj, psp_ = s_q.pop(0)
                            emit_pv(pqb, pj, psp_, *pvs[pqb])
                            if pj == NPAIR - 1:
                                emit_out(pqb, *pvs[pqb])
                # tail: drain last pair + final epilogue
                for pqb, pj, psp_ in s_q:
                    emit_pv(pqb, pj, psp_, *pvs[pqb])
                    if pj == NPAIR - 1:
                        emit_out(pqb, *pvs[pqb])

    nc.compile()
    return nc


def _get_nc():
    if "nc" not in _cache:
        _cache["nc"] = _build()
    return _cache["nc"]


def _host_prep(inputs):
    """Precompute weight layouts + packed constants (all fp32)."""
    wq = np.asarray(inputs["wq"], np.float32)
    wk = np.asarray(inputs["wk"], np.float32)
    wv = np.asarray(inputs["wv"], np.float32)
    wp = np.asarray(inputs["wp"], np.float32)
    M2 = wq.T @ wk  # [c'(q-side), c(k-side)]
    # lhsT layout [p, s, i] = M[i, p + 128 s], flattened to [P, 2*C]
    def lay(m):
        return np.ascontiguousarray(
            m.T.reshape(2, P, C).transpose(1, 0, 2).reshape(P, 2 * C))
    cbp = wp @ np.asarray(inputs["bv"], np.float32) + np.asarray(
        inputs["bp"], np.float32)
    cst = np.zeros((P, 22), np.float32)
    for i, v in enumerate((inputs["gn_gamma"], inputs["gn_beta"], cbp)):
        cst[:, 2 * i:2 * i + 2] = np.asarray(v, np.float32).reshape(2, P).T
    # ind[p, cc, g]: group-averaging weight; st2 holds per-channel SUMS over
    # n, so fold the full 1/(32*4096) mean divisor in here
    for cc in range(2):
        for j in range(4):
            g = cc * 4 + j
            cst[32 * j:32 * (j + 1), 6 + cc * G + g] = 1.0 / (32.0 * HW)
    cstT = np.zeros((G, C), np.float32)
    for g in range(G):
        cstT[g, 32 * g:32 * (g + 1)] = 1.0
    return {
        "m2t": lay(M2.T),
        "m3t": lay(wp @ wv),
        "cst": cst,
        "cstT": cstT,
    }


def _core_x(x, b, h):
    """Per-core x tensors: rolled so queries are cols 0:NQ, then laid out
    [p, cc, n] flattened to [P, 2*HW], in bf16 (stats+xn) and f32 (residual)."""
    import ml_dtypes

    xr = np.roll(x[b], -h * NQ, axis=1)           # [C, HW]
    xl = np.ascontiguousarray(
        xr.reshape(2, P, HW).transpose(1, 0, 2).reshape(P, 2 * HW))
    return {
        "xh": xl.astype(ml_dtypes.bfloat16),
        "xf": xl,
    }


def kernel(**inputs):
    from concourse.bass_utils import run_bass_kernel_spmd

    nc = _get_nc()
    x = np.ascontiguousarray(np.asarray(inputs["x"], dtype=np.float32)
                             ).reshape(4, C, HW)
    common = _host_prep(inputs)
    in_maps = []
    for p in range(NCORES):
        b, h = divmod(p, 2)
        m = dict(common)
        m.update(_core_x(x, b, h))
        in_maps.append(m)
    res = run_bass_kernel_spmd(nc, in_maps, list(range(NCORES)))
    out = np.empty((4, C, HW), np.float32)
    for p in range(NCORES):
        b, h = divmod(p, 2)
        out[b, :, h * NQ:(h + 1) * NQ] = res.results[p]["y"]
    return out.reshape(4, C, 64, 64)
